# revision 7
# baseline (speedup 1.0000x reference)
"""Trainium2 Bass kernel for nn_BPDecoder: logits = 1 - exp(-exp(sum_i R_i*||Z_i||^2)).

Strategy (8-core SPMD, row-sharded):
  - Pad N=500000 rows to 8 * 63488; core k takes rows [k*63488, (k+1)*63488).
  - Per core, 31 tiles of [128 partitions x (16 row-groups x 128 cols)]:
      ACT engine squares the tile, then 4 PE matmuls with the per-tile R
      column block [128, 16] as stationary weights accumulate
      C[q', (q, d)] += sum_p R[p, q'] * Z[p, q, d]^2 into one PSUM [16, 2048].
    The diagonal blocks q' == q of C hold the R-weighted row-norm partial sums.
  - Host extracts/sums the diagonals of the 8 small outputs and applies the
    scalar 1 - exp(-exp(s)).
"""

import sys

sys.path.insert(0, "/opt/trn_rl_repo")

import numpy as np

# The agent image lacks antenv.axon_hooks; recreate it so trace=True works
# (bass_utils imports it lazily for NTFF profiling under axon).
def _install_ntff_hook_shim():
    import types
    if "antenv.axon_hooks" in sys.modules:
        return
    mod = types.ModuleType("antenv.axon_hooks")
    state = {"hook": None}
    mod.set_axon_ntff_profile_hook = lambda h: state.__setitem__("hook", h)
    mod.get_axon_ntff_profile_hook = lambda: state["hook"]
    sys.modules["antenv.axon_hooks"] = mod
    try:
        sys.path.insert(0, "/root/.axon_site")
        from trn_agent_boot.trn_boot import _ntff_profile_via_ctypes
        state["hook"] = _ntff_profile_via_ctypes("/opt/axon/libaxon_pjrt.so")
    except Exception:
        pass


_install_ntff_hook_shim()

import concourse.bass as bass
import concourse.bacc as bacc
import concourse.mybir as mybir
from concourse.tile import TileContext
from concourse.bass_utils import run_bass_kernel_spmd

P = 128          # SBUF partitions
D = 128          # row length (feature dim)
Q = 16           # row-groups per tile
FREE = Q * D     # free elems per tile = 2048
T = 31           # tiles per core
NC_ROWS = T * Q * P   # 63488 rows per core
N_CORES = 8
N_FULL = 500000
MM_N = 512       # matmul moving-operand slice
NSLICES = FREE // MM_N
QS = Q // NSLICES     # q-groups per matmul slice

Z_DT = mybir.dt.float32
R_DT = mybir.dt.float32
S_DT = mybir.dt.float32   # dtype of the squared tile (matmul rhs)

_cache = {}


def _np_dt(dt):
    return mybir.dt.np(dt)


def _build():
    nc = bacc.Bacc(trn_type="TRN2")
    z = nc.declare_dram_parameter("z", [NC_ROWS, D], Z_DT, isOutput=False)
    r = nc.declare_dram_parameter("r", [NC_ROWS], R_DT, isOutput=False)
    out = nc.declare_dram_parameter("out", [Q, FREE], mybir.dt.float32, isOutput=True)

    z_tiles = z.rearrange("(t q p) d -> t p q d", p=P, q=Q)   # [T, 128, Q, D]
    r_cols = r.rearrange("(t q p) -> p (t q)", p=P, q=Q)      # [128, T*Q]

    with TileContext(nc) as tc:
        with (
            tc.tile_pool(name="zpool", bufs=8) as zpool,
            tc.tile_pool(name="spool", bufs=3) as spool,
            tc.tile_pool(name="singles", bufs=1) as singles,
            tc.tile_pool(name="ppool", bufs=1, space="PSUM") as ppool,
        ):
            r_dma = singles.tile([P, T * Q], R_DT)
            nc.sync.dma_start(out=r_dma[:], in_=r_cols)
            # Bounce R through ACT so matmuls depend on a single engine's
            # semaphore (Matmult has one sync-wait slot; ACT produces both
            # operands -> Tile merges the waits).
            r_sb = singles.tile([P, T * Q], R_DT)
            nc.scalar.copy(r_sb[:], r_dma[:])

            acc = ppool.tile([Q, FREE], mybir.dt.float32)

            for t in range(T):
                z_sb = zpool.tile([P, Q, D], Z_DT, tag="z")
                nc.sync.dma_start(out=z_sb[:], in_=z_tiles[t])
                s_sb = spool.tile([P, Q, D], S_DT, tag="s")
                nc.scalar.square(s_sb[:], z_sb[:])
                for sl in range(NSLICES):
                    nc.tensor.matmul(
                        acc[:, sl * MM_N:(sl + 1) * MM_N],
                        r_sb[:, t * Q:(t + 1) * Q],
                        s_sb[:, sl * QS:(sl + 1) * QS, :],
                        start=(t == 0),
                        stop=(t == T - 1),
                    )

            out_sb = singles.tile([Q, FREE], mybir.dt.float32)
            nc.vector.tensor_copy(out_sb[:], acc[:])
            nc.sync.dma_start(out=out[:], in_=out_sb[:])
    nc.compile()
    return nc


def _get_nc():
    if "nc" not in _cache:
        _cache["nc"] = _build()
    return _cache["nc"]


def _shard(Z, R):
    np_z = _np_dt(Z_DT)
    np_r = _np_dt(R_DT)
    ZP = np.zeros((N_CORES, NC_ROWS, D), dtype=np_z)
    ZP.reshape(-1, D)[:N_FULL] = Z.astype(np_z, copy=False)
    RP = np.zeros((N_CORES, NC_ROWS), dtype=np_r)
    RP.reshape(-1)[:N_FULL] = R.astype(np_r, copy=False)
    return [{"z": ZP[k], "r": RP[k]} for k in range(N_CORES)]


def _combine(results):
    idx = np.arange(Q)
    s = 0.0
    for res in results:
        C = np.asarray(res["out"], dtype=np.float64).reshape(Q, Q, D)
        s += C[idx, idx, :].sum()
    lam = np.exp(s)
    logits = 1.0 - np.exp(-lam)
    return np.float32(logits)


def _run(Z, R, trace=False):
    nc = _get_nc()
    in_maps = _shard(Z, R)
    return run_bass_kernel_spmd(nc, in_maps, core_ids=list(range(N_CORES)),
                                trace=trace)


def kernel(Z, R):
    assert Z.shape == (N_FULL, D) and R.shape == (N_FULL,)
    out = _run(np.asarray(Z), np.asarray(R), trace=False)
    return _combine(out.results)
